# revision 1
# baseline (speedup 1.0000x reference)
"""Causal self-attention (B=4, T=2048, C=1024, 16 heads) on 8 trn2 NeuronCores.

Sharding: core (b, g) = batch b in 0..3, head-group g in 0..1 (8 heads each).
Each core computes qkv projections for its 8 heads, causal attention, and a
partial output projection (its 512 rows of W_proj). Host sums the g=0/g=1
partials per batch (the "all-reduce after projection" done host-side).

Device layout choices (all matmuls contract over the partition dim):
  xT   [C=1024, T=2048]   x[b] transposed on host
  qT,kT[512, 2048]        head-major (h*64+d) rows; pair tile j holds heads 2j,2j+1
  v    [T, 8*65]          natural orientation + a ones column per head ->
                          P@V accumulation also yields softmax denominators
  S_T  [k, q] blocks      scores transposed so P_T feeds P@V directly as rhs
  OT   [512, 2048]        normalized attention output transposed = lhsT of proj

fp32r (~2x bf16 mantissa) for every matmul; exp without max-subtraction
(scores are ~N(0, 0.33^2), folded 1/sqrt(64) into W_q on host).
"""
import numpy as np
import concourse.bass as bass
import concourse.mybir as mybir
import concourse.tile as tile
from concourse import bacc
from concourse.bass_utils import run_bass_kernel_spmd

F32 = mybir.dt.float32
F32R = mybir.dt.float32r
EXPF = mybir.ActivationFunctionType.Exp

B, T, C = 4, 2048, 1024
NH, HD = 16, 64
HPC = 8             # heads per core
HDL = HPC * HD      # 512 local head-dims
KK = C // 128       # 8 contraction tiles
NT = T // 512       # 4 query/key 512-slices
MT = T // 128       # 16 key 128-blocks
MASK_NEG = -30000.0


def _emit_rep(nc, tc, io, r):
    xT_d, wq_d, wk_d, wv_d, wproj_d, masks_d, out_d = io
    sfx = f"_r{r}"

    def pool(name, **kw):
        return tc.tile_pool(name=name + sfx, **kw)

    with pool("pA", bufs=1) as pA:
        # persistent across stages: qT/kT pair tiles (0-3 q, 4-7 k), v tiles, ones
        qkT = [pA.tile([128, T], F32R, name=f"qkT{i}{sfx}", tag=f"qkT{i}") for i in range(8)]
        vp = [pA.tile([128, HPC, 65], F32R, name=f"vp{t}{sfx}", tag=f"vp{t}") for t in range(MT)]
        ones65f = pA.tile([65, 64], F32, name=f"ones65f{sfx}", tag="ones65f")
        ones65 = pA.tile([65, 64], F32R, name=f"ones65{sfx}", tag="ones65")
        ones8f = pA.tile([128, HPC], F32, name=f"ones8f{sfx}", tag="ones8f")
        nc.vector.memset(ones65f[:], 1.0)
        nc.vector.tensor_copy(ones65[:], ones65f[:])
        nc.vector.memset(ones8f[:], 1.0)

        # ---------------- stage 1: qkv projections ----------------
        with (
            pool("s1x", bufs=1) as px,
            pool("s1st", bufs=1) as pst,
            pool("s1ps", bufs=1, space="PSUM") as pps,
        ):
            xTr = [px.tile([128, T], F32R, name=f"xTr{kk}{sfx}", tag=f"xTr{kk}") for kk in range(KK)]
            for kk in range(KK):
                xst = pst.tile([128, T], F32, name=f"xst{kk}{sfx}", tag="xst", bufs=2)
                nc.sync.dma_start(xst[:], xT_d[kk * 128:(kk + 1) * 128, :])
                nc.vector.tensor_copy(xTr[kk][:], xst[:])
            wvr = [px.tile([128, HDL], F32R, name=f"wvr{kk}{sfx}", tag=f"wvr{kk}") for kk in range(KK)]
            for kk in range(KK):
                wvst = pst.tile([128, HDL], F32, name=f"wvst{kk}{sfx}", tag="wvst", bufs=2)
                nc.sync.dma_start(wvst[:], wv_d[kk * 128:(kk + 1) * 128, :])
                nc.vector.tensor_copy(wvr[kk][:], wvst[:])

            # q and k, transposed form: qT = wq.T @ x.T (lhsT = wq slice, rhs = xT)
            for m in range(8):
                mat_d = wq_d if m < 4 else wk_d
                mcol = (m % 4) * 128
                wr = []
                for kk in range(KK):
                    wst = pst.tile([128, 128], F32, name=f"wst{m}_{kk}{sfx}", tag="wst", bufs=4)
                    nc.sync.dma_start(wst[:], mat_d[kk * 128:(kk + 1) * 128, mcol:mcol + 128])
                    w1 = pst.tile([128, 128], F32R, name=f"wr{m}_{kk}{sfx}", tag="wr", bufs=16)
                    nc.vector.tensor_copy(w1[:], wst[:])
                    wr.append(w1)
                for n in range(NT):
                    psqk = pps.tile([128, 512], F32, name=f"psqk{m}_{n}{sfx}", tag="psqk", bufs=3)
                    for kk in range(KK):
                        nc.tensor.matmul(psqk[:], wr[kk][:], xTr[kk][:, n * 512:(n + 1) * 512],
                                         start=(kk == 0), stop=(kk == KK - 1))
                    nc.vector.tensor_copy(qkT[m][:, n * 512:(n + 1) * 512], psqk[:])

            # v, natural form: v = x @ wv (lhsT = xT slice, rhs = wv)
            for t in range(MT):
                psv = pps.tile([128, HDL], F32, name=f"psv{t}{sfx}", tag="psv", bufs=3)
                for kk in range(KK):
                    nc.tensor.matmul(psv[:], xTr[kk][:, t * 128:(t + 1) * 128], wvr[kk][:],
                                     start=(kk == 0), stop=(kk == KK - 1))
                nc.vector.tensor_copy(vp[t][:, :, 0:64], psv[:].rearrange("p (h d) -> p h d", h=HPC))
                nc.vector.tensor_copy(vp[t][:, :, 64], ones8f[:])

        # ---------------- stage 2: causal attention ----------------
        with pool("pOT", bufs=1) as po, pool("pMK", bufs=1) as pm:
            OT = [po.tile([128, T], F32R, name=f"OT{j}{sfx}", tag=f"OT{j}") for j in range(4)]
            maskt = pm.tile([128, 4, 512], F32, name=f"maskt{sfx}", tag="maskt")
            nc.sync.dma_start(maskt[:], masks_d[:])
            with (
                pool("s2", bufs=1) as p2,
                pool("s2ps", bufs=1, space="PSUM") as pps2,
            ):
                for j in range(4):          # head pairs (2j, 2j+1)
                    qTj, kTj = qkT[j], qkT[4 + j]
                    for ni in range(NT):
                        last = 4 * ni + 3
                        ot0 = pps2.tile([65, 512], F32, name=f"otps0_{j}_{ni}{sfx}", tag="otps0", bufs=1)
                        ot1 = pps2.tile([65, 512], F32, name=f"otps1_{j}_{ni}{sfx}", tag="otps1", bufs=1)
                        for mi in range(last + 1):
                            s0 = pps2.tile([128, 512], F32, name=f"s0_{j}_{ni}_{mi}{sfx}", tag="s0", bufs=2)
                            s1 = pps2.tile([128, 512], F32, name=f"s1_{j}_{ni}_{mi}{sfx}", tag="s1", bufs=2)
                            nc.tensor.matmul(s0[:], kTj[0:64, mi * 128:(mi + 1) * 128],
                                             qTj[0:64, ni * 512:(ni + 1) * 512],
                                             start=True, stop=True, tile_position=(0, 0))
                            nc.tensor.matmul(s1[:], kTj[64:128, mi * 128:(mi + 1) * 128],
                                             qTj[64:128, ni * 512:(ni + 1) * 512],
                                             start=True, stop=True, tile_position=(64, 0))
                            d = mi - 4 * ni
                            if d >= 0:      # diagonal block: apply causal mask
                                nc.vector.tensor_add(s0[:], s0[:], maskt[:, d, :])
                                nc.vector.tensor_add(s1[:], s1[:], maskt[:, d, :])
                            p0 = p2.tile([128, 512], F32R, name=f"p0_{j}_{ni}_{mi}{sfx}", tag="p0", bufs=3)
                            p1 = p2.tile([128, 512], F32R, name=f"p1_{j}_{ni}_{mi}{sfx}", tag="p1", bufs=3)
                            nc.scalar.activation(p0[:], s0[:], EXPF)
                            nc.scalar.activation(p1[:], s1[:], EXPF)
                            nc.tensor.matmul(ot0[:], vp[mi][:, 2 * j, :], p0[:],
                                             start=(mi == 0), stop=(mi == last))
                            nc.tensor.matmul(ot1[:], vp[mi][:, 2 * j + 1, :], p1[:],
                                             start=(mi == 0), stop=(mi == last))
                        for h01, otp in ((0, ot0), (1, ot1)):
                            dn = p2.tile([65, 512], F32R, name=f"dn{j}_{ni}_{h01}{sfx}", tag="dn", bufs=2)
                            nc.vector.reciprocal(dn[64:65, :], otp[64:65, :])
                            bcp = pps2.tile([64, 512], F32, name=f"bcp{j}_{ni}_{h01}{sfx}", tag="bcp", bufs=2)
                            nc.tensor.matmul(bcp[:], ones65[64:65, :], dn[64:65, :], start=True, stop=True)
                            bcs = p2.tile([64, 512], F32, name=f"bcs{j}_{ni}_{h01}{sfx}", tag="bcs", bufs=2)
                            nc.scalar.copy(bcs[:], bcp[:])
                            nc.vector.tensor_mul(OT[j][h01 * 64:(h01 + 1) * 64, ni * 512:(ni + 1) * 512],
                                                 otp[0:64, :], bcs[:])

            # ---------------- stage 3: output projection ----------------
            with (
                pool("s3", bufs=1) as p3,
                pool("s3ps", bufs=1, space="PSUM") as pps3,
            ):
                wpr = [p3.tile([128, C], F32R, name=f"wpr{kt}{sfx}", tag=f"wpr{kt}") for kt in range(4)]
                for kt in range(4):
                    wpst = p3.tile([128, C], F32, name=f"wpst{kt}{sfx}", tag="wpst", bufs=2)
                    nc.sync.dma_start(wpst[:], wproj_d[kt * 128:(kt + 1) * 128, :])
                    nc.vector.tensor_copy(wpr[kt][:], wpst[:])
                for jm in range(MT):
                    for n2 in range(2):
                        pso = pps3.tile([128, 512], F32, name=f"pso{jm}_{n2}{sfx}", tag="pso", bufs=4)
                        for kt in range(4):
                            nc.tensor.matmul(pso[:], OT[kt][:, jm * 128:(jm + 1) * 128],
                                             wpr[kt][:, n2 * 512:(n2 + 1) * 512],
                                             start=(kt == 0), stop=(kt == 3))
                        osb = p3.tile([128, 512], F32, name=f"osb{jm}_{n2}{sfx}", tag="osb", bufs=4)
                        nc.vector.tensor_copy(osb[:], pso[:])
                        nc.sync.dma_start(out_d[jm * 128:(jm + 1) * 128, n2 * 512:(n2 + 1) * 512], osb[:])


def build(reps=1):
    nc = bacc.Bacc(None, target_bir_lowering=False, debug=False)
    xT_d = nc.declare_dram_parameter("xT", [C, T], F32, isOutput=False)
    wq_d = nc.declare_dram_parameter("wq", [C, HDL], F32, isOutput=False)
    wk_d = nc.declare_dram_parameter("wk", [C, HDL], F32, isOutput=False)
    wv_d = nc.declare_dram_parameter("wv", [C, HDL], F32, isOutput=False)
    wproj_d = nc.declare_dram_parameter("wproj", [HDL, C], F32, isOutput=False)
    masks_d = nc.declare_dram_parameter("masks", [128, 4, 512], F32, isOutput=False)
    out_d = nc.declare_dram_parameter("out", [T, C], F32, isOutput=True)
    io = (xT_d, wq_d, wk_d, wv_d, wproj_d, masks_d, out_d)
    with tile.TileContext(nc) as tc:
        with nc.allow_low_precision(reason="fp32r attention pipeline"):
            for r in range(reps):
                _emit_rep(nc, tc, io, r)
    nc.compile()
    return nc


def make_masks():
    r = np.arange(128)[:, None]
    c = np.arange(512)[None, :]
    m = np.empty((128, 4, 512), dtype=np.float32)
    for d in range(4):
        m[:, d, :] = np.where(128 * d + r <= c, 0.0, MASK_NEG)
    return m


def make_in_maps(x, W_attn, W_proj):
    x = np.asarray(x, dtype=np.float32)
    W_attn = np.asarray(W_attn, dtype=np.float32)
    W_proj = np.asarray(W_proj, dtype=np.float32)
    masks = make_masks()
    scale = 1.0 / np.sqrt(HD)
    in_maps = []
    for core in range(8):
        b, g = core // 2, core % 2
        cols = slice(g * HDL, (g + 1) * HDL)
        in_maps.append({
            "xT": np.ascontiguousarray(x[b].T),
            "wq": np.ascontiguousarray(W_attn[:, g * HDL:(g + 1) * HDL]) * scale,
            "wk": np.ascontiguousarray(W_attn[:, C + g * HDL:C + (g + 1) * HDL]),
            "wv": np.ascontiguousarray(W_attn[:, 2 * C + g * HDL:2 * C + (g + 1) * HDL]),
            "wproj": np.ascontiguousarray(W_proj[g * HDL:(g + 1) * HDL, :]),
            "masks": masks,
        })
    return in_maps


_NC_CACHE = {}


def kernel(x, W_attn, W_proj):
    if "nc" not in _NC_CACHE:
        _NC_CACHE["nc"] = build(reps=1)
    nc = _NC_CACHE["nc"]
    in_maps = make_in_maps(x, W_attn, W_proj)
    res = run_bass_kernel_spmd(nc, in_maps, list(range(8)), trace=False)
    out = np.empty((B, T, C), dtype=np.float32)
    for b in range(B):
        out[b] = res.results[2 * b]["out"] + res.results[2 * b + 1]["out"]
    return out
